# revision 63
# baseline (speedup 1.0000x reference)
"""Multi-head attention forward on 8 Trainium2 NeuronCores (Bass/Tile).

Problem: B=4, L=2048, D=1024, H=16 heads, DV=64.
  out = softmax((x_q Wq^T)(x_k Wk^T)^T / sqrt(DV)) (x_v Wv^T) Wc^T + biases

Sharding (8 cores): core c handles batch b = c//2 and head-group g = c%2
(8 heads = 512 of the 1024 projection columns). Each core produces a
full-shape [L, D] partial of the output projection; the host sums the two
partials per batch and adds bc.

All device data is bf16 (PSUM accumulation fp32); inputs are converted on
the host. Per-core pipeline:
  A1. V projection -> vext [128, 8, 65] bf16 per l-tile (ones column per
      head gives the softmax denominator through the AV matmul).
  A2. Q/K projections per head-pair -> qt/kt [128(2x64 dims), 2048] bf16.
      Pair-0 head uses dedicated 128-col weight slices so the first
      matmuls only wait on ~3.5MB of DMA (not the bulk weights).
  B.  Per (head, q-half) unit: 16 k-tiles of scores^T [k=128, q=1024] in
      PSUM -> ACT exp (scale=1/8) -> ex bf16; then AV flipped:
      out[q=128, 65] chains (lhsT=ex slice, rhs=vext) -> per-partition
      denominator -> reciprocal_approx_fast + tensor_scalar normalize
      into att_s [128, 64, 16]. Per head-pair, SBUF->SBUF DMA xbar
      transposes produce attnt [64, 2048] (= attn^T) for stage C.
      Pair 3 runs its units qc-major and transposes in q-halves so the
      final output projection can start before the last unit retires.
  C.  Output projection out[l, 1024] accumulated over the 4 head pairs;
      pairs 0-2 staged in bf16, final pass adds pair 3 via an identity
      matmul. lt 0-7 of the final pass interleave into unit 15.
"""

import itertools
from contextlib import ExitStack

import numpy as np

import concourse.bacc as bacc
import concourse.mybir as mybir
from concourse.tile import TileContext
from concourse.bass_utils import run_bass_kernel_spmd

try:
    from ml_dtypes import bfloat16 as np_bf16
except ImportError:  # pragma: no cover
    import jax.numpy as jnp
    np_bf16 = jnp.bfloat16

B, L, D, H = 4, 2048, 1024, 16
DV = 64
HPC = 8           # heads per core
OC = HPC * DV     # 512 projection cols per core
NCORES = 8

F32 = mybir.dt.float32
BF16 = mybir.dt.bfloat16
EXP = mybir.ActivationFunctionType.Exp
MULT = mybir.AluOpType.mult
ADD = mybir.AluOpType.add

NI = D // 128    # 8 contraction tiles for projections
NM = OC // 128   # 4 head pairs
NLT = L // 128   # 16 l/k tiles
QW = 1024        # q-half width in stage B

_CACHE = {}


def _build():
    nc = bacc.Bacc("TRN2", target_bir_lowering=False, debug=False,
                   num_devices=NCORES)

    # Inputs are host-repacked so every DMA descriptor is 2-8KB contiguous
    # per partition (1KB descriptors saturate the 16 queues at half rate):
    #   XT*[p, lc, i, c] = x^T[i*128+p, lc*512+c]
    #   WQH/WKH[p, i, c] = W^T[i*128+p, c]       (head cols 0:128)
    #   WQB/WKB[p, i, c] = W^T[i*128+p, 128+c]   (cols 128:512)
    #   WVT[p, i, c] = Wv^T[i*128+p, c];  WCT[p, m, c] = Wc^T[m*128+p, c]
    xtq = nc.dram_tensor("XTQ", [128, 4, NI, 512], BF16, kind="ExternalInput")
    xtk = nc.dram_tensor("XTK", [128, 4, NI, 512], BF16, kind="ExternalInput")
    xtv = nc.dram_tensor("XTV", [128, 4, NI, 512], BF16, kind="ExternalInput")
    wqh = nc.dram_tensor("WQH", [128, NI, 128], BF16, kind="ExternalInput")
    wqb = nc.dram_tensor("WQB", [128, NI, OC - 128], BF16,
                         kind="ExternalInput")
    wkh = nc.dram_tensor("WKH", [128, NI, 128], BF16, kind="ExternalInput")
    wkb = nc.dram_tensor("WKB", [128, NI, OC - 128], BF16,
                         kind="ExternalInput")
    wvt = nc.dram_tensor("WVT", [128, NI, OC], BF16, kind="ExternalInput")
    wct = nc.dram_tensor("WCT", [128, NM, D], BF16, kind="ExternalInput")
    bqd = nc.dram_tensor("BQ", [OC], F32, kind="ExternalInput")
    bkd = nc.dram_tensor("BK", [OC], F32, kind="ExternalInput")
    bvd = nc.dram_tensor("BV", [OC], F32, kind="ExternalInput")
    eyed = nc.dram_tensor("EYE", [128, 128], BF16, kind="ExternalInput")
    out = nc.dram_tensor("OUT", [L, D], BF16, kind="ExternalOutput")

    # Fillers (projections, C passes) get +1M priority so the greedy tile
    # scheduler only runs them when the exp-paced backbone stalls, instead
    # of monopolizing the PE in long ready-ahead streaks.
    FILLER = -1_000_000

    with TileContext(nc) as tc:
        with (
            tc.tile_pool(name="const", bufs=1) as const_pool,
            tc.tile_pool(name="wcp", bufs=1) as wc_pool,
            tc.tile_pool(name="qkt", bufs=6) as qkt_pool,
            tc.tile_pool(name="vext", bufs=NLT) as vext_pool,
            tc.tile_pool(name="ex", bufs=34) as ex_pool,
            tc.tile_pool(name="atts", bufs=2) as atts_pool,
            tc.tile_pool(name="attnt", bufs=NM) as attnt_pool,
            tc.tile_pool(name="rcp", bufs=4) as rcp_pool,
            tc.tile_pool(name="ob", bufs=4) as ob_pool,
            tc.tile_pool(name="st", bufs=2, space="PSUM") as st_pool,
            tc.tile_pool(name="ap", bufs=2, space="PSUM") as ap_pool,
            tc.tile_pool(name="av", bufs=2, space="PSUM") as av_pool,
        ):
            astack = ExitStack()
            wv_pool = astack.enter_context(tc.tile_pool(name="wvp", bufs=1))
            wq_pool = astack.enter_context(tc.tile_pool(name="wqp", bufs=1))
            wk_pool = astack.enter_context(tc.tile_pool(name="wkp", bufs=1))
            xt_pool = astack.enter_context(tc.tile_pool(name="xt", bufs=4))

            # ---- constants / weights ----
            qb_tile = const_pool.tile([128, NM], F32, tag="bq", name="bqt")
            nc.sync.dma_start(
                out=qb_tile, in_=bqd[:].rearrange("(m p) -> p m", p=128))
            kb_tile = const_pool.tile([128, NM], F32, tag="bk", name="bkt")
            nc.sync.dma_start(
                out=kb_tile, in_=bkd[:].rearrange("(m p) -> p m", p=128))
            vbias = const_pool.tile([128, OC], F32, tag="bv", name="bvt")
            onesf = const_pool.tile([128, HPC], F32, tag="ones", name="ones")
            nc.vector.memset(onesf, 1.0)
            eye = const_pool.tile([128, 128], BF16, tag="eye", name="eye")

            wq_head = wq_pool.tile([128, NI, 128], BF16, tag="wqh",
                                   name="wq_h")
            wq_bulk = wq_pool.tile([128, NI, OC - 128], BF16, tag="wqb",
                                   name="wq_b")
            wk_head = wk_pool.tile([128, NI, 128], BF16, tag="wkh",
                                   name="wk_h")
            wk_bulk = wk_pool.tile([128, NI, OC - 128], BF16, tag="wkb",
                                   name="wk_b")

            def wsel_q(i, m):
                return (wq_head[:, i, :] if m == 0
                        else wq_bulk[:, i, (m - 1) * 128:m * 128])

            def wsel_k(i, m):
                return (wk_head[:, i, :] if m == 0
                        else wk_bulk[:, i, (m - 1) * 128:m * 128])

            wv_all = wv_pool.tile([128, NI, OC], BF16, tag="wv", name="wv_a")
            wv_tiles = [wv_all[:, i, :] for i in range(NI)]
            wc_all = wc_pool.tile([128, NM, D], BF16, tag="wc", name="wc_a")
            wc_tiles = [wc_all[:, i, :] for i in range(NM)]

            # qt/kt allocated lazily from a 6-deep ring: pair 3's tiles reuse
            # pair 0's buffers (dead after unit 3).
            qt = {}
            kt = {}

            def get_qkt(d, m):
                if m not in d:
                    pfx = "qt" if d is qt else "kt"
                    d[m] = qkt_pool.tile([128, L], BF16, tag="qkt",
                                         name=f"{pfx}{m}")
                return d[m]
            vext = [vext_pool.tile([128, HPC, DV + 1], BF16, tag="vext",
                                   name=f"vext{i}")
                    for i in range(NLT)]

            # ---- stage A1 generator: V projection (128 mms + DVE) ----
            # Yields BEFORE each matmul (except the first) so the pull that
            # consumes the last matmul also emits the trailing DVE writes.
            def a1_gen():
                first = [True]

                def tick():
                    if first[0]:
                        first[0] = False
                        return iter(())
                    return iter((None,))

                xas = {}

                def load_v(lc):
                    xa = xt_pool.tile([128, NI, 512], BF16, tag="xt",
                                      name="xv")
                    nc.sync.dma_start(out=xa, in_=xtv[:, lc, :, :])
                    xas[lc] = xa

                load_v(0)
                load_v(1)
                for lc in range(4):
                    if lc + 2 < 4:
                        load_v(lc + 2)
                    xa = xas.pop(lc)
                    for ls in range(4):
                        ps = ap_pool.tile([128, 512], F32, tag="ap",
                                          name="psv")
                        for i in range(NI):
                            yield from tick()
                            nc.tensor.matmul(
                                ps,
                                lhsT=xa[:, i, ls * 128:(ls + 1) * 128],
                                rhs=wv_tiles[i],
                                start=(i == 0), stop=(i == NI - 1))
                        lt = lc * 4 + ls
                        nc.vector.tensor_add(
                            vext[lt][:, :, 0:DV],
                            ps.rearrange("p (h d) -> p h d", h=HPC),
                            vbias.rearrange("p (h d) -> p h d", h=HPC))
                        nc.vector.tensor_copy(vext[lt][:, :, DV], onesf)

            def a2_chunk(wsel, dst, btile, m, xa):
                ps = ap_pool.tile([128, 512], F32, tag="ap", name="psp")
                lc = xa[1]
                for i in range(NI):
                    yield
                    nc.tensor.matmul(
                        ps,
                        lhsT=wsel(i, m),
                        rhs=xa[0][:, i, :],
                        start=(i == 0), stop=(i == NI - 1))
                nc.vector.tensor_scalar(
                    out=dst[:, lc * 512:(lc + 1) * 512],
                    in0=ps,
                    scalar1=btile[:, m:m + 1],
                    scalar2=None, op0=ADD)

            def xload(xsrc, lc, name):
                xa = xt_pool.tile([128, NI, 512], BF16, tag="xt", name=name)
                nc.sync.dma_start(out=xa, in_=xsrc[:, lc, :, :])
                return (xa, lc)

            def run(gen):
                for _ in gen:
                    pass

            tail_xas = []

            def a2_pair0_head():
                # Critical path: only the 128-col (m=0) weight slices +
                # the first three x chunks gate the first matmuls.
                xk0 = xload(xtk, 0, "xk0")
                nc.sync.dma_start(out=wk_head, in_=wkh[:, :, :])
                xq0 = xload(xtq, 0, "xq0")
                nc.sync.dma_start(out=wq_head, in_=wqh[:, :, :])
                xq1 = xload(xtq, 1, "xq1")
                run(a2_chunk(wsel_k, get_qkt(kt, 0), kb_tile, 0, xk0))
                run(a2_chunk(wsel_q, get_qkt(qt, 0), qb_tile, 0, xq0))
                run(a2_chunk(wsel_q, qt[0], qb_tile, 0, xq1))
                # next-critical: the k chunks feeding unit-0 scores k=4..15
                tail_xas.extend(xload(xtk, lc, "xk0t") for lc in (1, 2, 3))
                # bulk weights (cols 128+ needed from ui=2 onward)
                nc.sync.dma_start(out=wk_bulk, in_=wkb[:, :, :])
                nc.sync.dma_start(out=wq_bulk, in_=wqb[:, :, :])

            def a2_pair0_tail():
                first = [True]
                chunks = [(xtk, 1, wsel_k, kt[0], kb_tile),
                          (xtk, 2, wsel_k, kt[0], kb_tile),
                          (xtk, 3, wsel_k, kt[0], kb_tile),
                          (xtq, 2, wsel_q, qt[0], qb_tile),
                          (xtq, 3, wsel_q, qt[0], qb_tile)]
                xas = list(tail_xas)
                for ci, (xsrc, lc, wsel, dst, btile) in enumerate(chunks):
                    if ci + 3 < len(chunks):
                        xas.append(xload(chunks[ci + 3][0],
                                         chunks[ci + 3][1], "xt0t"))
                    g = a2_chunk(wsel, dst, btile, 0, xas[ci])
                    for _ in g:
                        if first[0]:
                            first[0] = False
                        else:
                            yield

            # ---- A2 generator: pair 1 alone, then pairs (2,3) sharing each
            # x chunk so those 16MB of x are loaded once, not twice. ----
            def a2_gen(ms):
                first = [True]

                def tick():
                    if first[0]:
                        first[0] = False
                        return iter(())
                    return iter((None,))

                xas = {}

                def load_x(xsrc, lc):
                    xa = xt_pool.tile([128, NI, 512], BF16, tag="xt",
                                      name="xp")
                    nc.sync.dma_start(out=xa, in_=xsrc[:, lc, :, :])
                    xas[(id(xsrc), lc)] = xa

                # chunk order: q01 (next pair's qc=0 scores), k full (its
                # scores k=0..15 by the pair boundary), q23 (its qc=1 unit).
                chunks = [(xtq, 0), (xtq, 1), (xtk, 0), (xtk, 1),
                          (xtk, 2), (xtk, 3), (xtq, 2), (xtq, 3)]
                for m in ms:
                    get_qkt(qt, m)
                    get_qkt(kt, m)
                load_x(*chunks[0])
                load_x(*chunks[1])
                for ci, (xsrc, lc) in enumerate(chunks):
                    if ci + 2 < len(chunks):
                        load_x(*chunks[ci + 2])
                    if ci == 5 and 3 in ms:
                        nc.sync.dma_start(out=wc_all, in_=wct[:, :, :])
                    xa = xas.pop((id(xsrc), lc))
                    wsel = wsel_q if xsrc is xtq else wsel_k
                    dst_l = qt if xsrc is xtq else kt
                    btile = qb_tile if xsrc is xtq else kb_tile
                    for m in ms:
                        ps = ap_pool.tile([128, 512], F32, tag="ap",
                                          name="psp")
                        for i in range(NI):
                            yield from tick()
                            nc.tensor.matmul(
                                ps,
                                lhsT=wsel(i, m),
                                rhs=xa[:, i, :],
                                start=(i == 0), stop=(i == NI - 1))
                        nc.vector.tensor_scalar(
                            out=dst_l[m][:, lc * 512:(lc + 1) * 512],
                            in0=ps,
                            scalar1=btile[:, m:m + 1],
                            scalar2=None, op0=ADD)

            def fprio():
                return tc.high_priority(offset=FILLER)

            def drain(gen, filler=True):
                if gen is None:
                    return
                if filler:
                    with fprio():
                        for _ in gen:
                            pass
                else:
                    for _ in gen:
                        pass

            # ---- pre-B: A2(0) head, then non-critical consts + V weights ----
            a2_pair0_head()
            nc.sync.dma_start(
                out=vbias, in_=bvd[:].unsqueeze(0).to_broadcast((128, OC)))
            nc.sync.dma_start(out=eye, in_=eyed[:, :])
            nc.sync.dma_start(out=wv_all, in_=wvt[:, :, :])

            # ---- stage B ----
            # Units: (m, h2, qc); AV of unit u runs after scores of u+1.
            # Pair 3 runs qc-major so the q-halves of attnt[3] complete (and
            # transpose) separately, letting stage C start before the end.
            units = []
            for m in range(NM):
                order = (((0, 0), (0, 1), (1, 0), (1, 1)) if m < 3
                         else ((0, 0), (1, 0), (0, 1), (1, 1)))
                for h2, qc in order:
                    units.append((m, h2, qc))

            cstage = {}     # opened after stage-A pools close

            def c_gen(dts, first):
                # stage-C partial pass: psum chain over `dts`, then DVE
                # copy/accumulate into the bf16 staging tile.
                firstmm = [True]

                def tick():
                    if firstmm[0]:
                        firstmm[0] = False
                        return iter(())
                    return iter((None,))

                for lt in range(NLT):
                    for nck in range(2):
                        ps = ap_pool.tile([128, 512], F32, tag="ap",
                                          name="psc")
                        for di, dt in enumerate(dts):
                            yield from tick()
                            nc.tensor.matmul(
                                ps,
                                lhsT=attnt[dt][:, lt * 128:(lt + 1) * 128],
                                rhs=wc_tiles[dt][:, nck * 512:(nck + 1) * 512],
                                start=(di == 0), stop=(di == len(dts) - 1))
                        sl = cstage["t"][:, lt, nck * 512:(nck + 1) * 512]
                        if first:
                            nc.vector.tensor_copy(sl, ps)
                        else:
                            nc.vector.tensor_add(sl, ps, sl)

            def cfinal_gen(lts, act_half=False, pads=0, use_av=True):
                # final stage-C pass: pair 3 + staged partial via identity
                # matmul; PSUM drained by DVE (and ACT when it is free).
                # use_av=False keeps it off av_pool while AV tiles of the
                # last units still occupy that ring (circular-wait hazard).
                for _ in range(pads):
                    yield
                for lt in lts:
                    ob = ob_pool.tile([128, QW], BF16, tag="ob", name="ob")
                    for nck in range(2):
                        pool, tg = ((ap_pool, "ap") if (nck == 0 or not use_av)
                                    else (av_pool, "av"))
                        ps = pool.tile([128, 512], F32, tag=tg, name="psf")
                        nc.tensor.matmul(
                            ps,
                            lhsT=attnt[3][:, lt * 128:(lt + 1) * 128],
                            rhs=wc_tiles[3][:, nck * 512:(nck + 1) * 512],
                            start=True, stop=False)
                        yield
                        nc.tensor.matmul(
                            ps,
                            lhsT=eye,
                            rhs=cstage["t"][:, lt, nck * 512:(nck + 1) * 512],
                            start=False, stop=True)
                        if nck == 0:
                            nc.vector.tensor_copy(ob[:, 0:512], ps)
                        elif act_half:
                            nc.scalar.copy(ob[:, 512:QW], ps)
                        else:
                            nc.vector.tensor_copy(ob[:, 512:QW], ps)
                        # store each half as soon as its copy lands
                        nc.sync.dma_start(
                            out=out[lt * 128:(lt + 1) * 128,
                                    nck * 512:(nck + 1) * 512],
                            in_=ob[:, nck * 512:(nck + 1) * 512])
                        yield

            exs = {}        # unit idx -> list of 16 ex tiles
            att_s = {}      # pair -> staging tile
            avt = {}        # (ui, qsg) -> av psum region view
            attnt = {}

            def emit_chain(pu, j):
                # one AV accumulation chain (16 k-steps) for q-subtile j.
                # Chains are sequential per PSUM bank: interleaving open
                # accumulation groups within a bank clobbers earlier regions.
                m, h2, qc = units[pu]
                h = m * 2 + h2
                qsg, qs = j // 4, j % 4
                if (pu, qsg) not in avt:
                    t = av_pool.tile([128, 512], F32, tag="av", name="av")
                    avt[(pu, qsg)] = t[:, 0:4 * (DV + 1)].rearrange(
                        "p (a b) -> p a b", b=DV + 1)
                av = avt[(pu, qsg)]
                ex_list = exs[pu]
                qq = (qsg * 4 + qs) * 128
                for k in range(NLT):
                    nc.tensor.matmul(
                        av[:, qs, :],
                        lhsT=ex_list[k][:, qq:qq + 128],
                        rhs=vext[k][:, h, :],
                        start=(k == 0), stop=(k == NLT - 1))

            def emit_norm(pu, qsg):
                m, h2, qc = units[pu]
                av = avt.pop((pu, qsg))
                ast = att_s[m]
                rc = rcp_pool.tile([128, 4], F32, tag="rc", name="rc")
                nc.vector.reciprocal_approx_fast(out=rc, in_=av[:, :, DV])
                for qs in range(4):
                    lt = qc * 8 + qsg * 4 + qs
                    nc.vector.tensor_scalar(
                        out=ast[:, lt, h2 * DV:(h2 + 1) * DV],
                        in0=av[:, qs, 0:DV],
                        scalar1=rc[:, qs:qs + 1],
                        scalar2=None, op0=MULT)
                if m == 3 and h2 == 1:
                    # qc-major order: 4-lt quarter of pair 3 complete; per-lt
                    # transposes let the final C pass chase each norm write.
                    if 3 not in attnt:
                        attnt[3] = attnt_pool.tile(
                            [128, L], BF16, tag="attnt", name="attnt3")
                    lt0 = qc * 8 + qsg * 4
                    for lt in range(lt0, lt0 + 4):
                        dst = attnt[3][:, lt * 128:(lt + 1) * 128].rearrange(
                            "p (t q) -> p t q", t=1)
                        nc.sync.dma_start_transpose(
                            dst, ast[:, lt:lt + 1, :])
                    if qc == 1 and qsg == 1:
                        del att_s[3]
                if qsg == 1:
                    exs.pop(pu)
                    if m < 3 and h2 == 1 and qc == 1:
                        # pair complete: xbar transpose
                        # out[fm, b, q] = in[q, b*128+fm], fm = h2*64+dv
                        attnt[m] = attnt_pool.tile(
                            [128, L], BF16, tag="attnt", name=f"attnt{m}")
                        dst = attnt[m][:, :].rearrange(
                            "p (t q) -> p t q", t=NLT)
                        nc.sync.dma_start_transpose(dst, ast)
                        del att_s[m]

            # prev-unit AV schedules: slot -> actions (c<j> chain, n<g> norm)
            # Nothing at "end": bunching c7+n1 there delays the next unit's
            # first scores and opens a ~1.2us ACT bubble per unit boundary.
            SPREAD = {1: ["c0"], 3: ["c1"], 5: ["c2"], 7: ["c3"],
                      8: ["n0"], 9: ["c4"], 11: ["c5"], 13: ["c6"],
                      15: ["c7", "n1"]}
            SQUEEZE = {10: ["c0"], 11: ["c1"], 12: ["c2", "c3"],
                       13: ["n0", "c4"], 14: ["c5", "c6"],
                       15: ["c7", "n1"]}

            def do_actions(prev, acts):
                for a in acts:
                    if a[0] == "c":
                        emit_chain(prev, int(a[1]))
                    else:
                        emit_norm(prev, int(a[1]))

            def emit_unit(ui, intl_gen, n_intl):
                m, h2, qc = units[ui]
                off = h2 * DV
                prev = ui - 1 if ui > 0 else None
                sched = SQUEEZE if ui == 1 else SPREAD
                if m not in att_s:
                    att_s[m] = atts_pool.tile(
                        [128, NLT, 128], BF16, tag="atts", name=f"atts{m}")
                ex_list = []
                exs[ui] = ex_list
                for k in range(NLT):
                    if intl_gen is not None:
                        with fprio():
                            for _ in range(n_intl):
                                if next(intl_gen, "END") == "END":
                                    intl_gen = None
                                    break
                    if prev is not None:
                        do_actions(prev, sched.get(k, ()))
                    st = st_pool.tile([128, QW], F32, tag="st", name="st")
                    for j in range(2):
                        nc.tensor.matmul(
                            st[:, j * 512:(j + 1) * 512],
                            lhsT=kt[m][off:off + DV, k * 128:(k + 1) * 128],
                            rhs=qt[m][off:off + DV,
                                      qc * QW + j * 512:qc * QW + (j + 1) * 512],
                            start=True, stop=True)
                    ex = ex_pool.tile([128, QW], BF16, tag="ex", name="ex")
                    nc.scalar.activation(out=ex, in_=st, func=EXP, scale=0.125)
                    ex_list.append(ex)
                if prev is not None:
                    do_actions(prev, sched.get("end", ()))
                return intl_gen

            intl = itertools.chain(a2_pair0_tail(), a1_gen())
            n_per_slot = 8
            for ui in range(len(units)):
                if ui == 2:
                    drain(intl)
                    intl = a2_gen((1,))
                    n_per_slot = 2
                elif ui == 4:
                    drain(intl)
                    intl = a2_gen((2, 3))
                    n_per_slot = 2
                elif ui == 8:
                    drain(intl)
                    intl = None
                elif ui == 10:
                    astack.close()
                    cs_cm = tc.tile_pool(name="cst", bufs=1)
                    cs_pool = cs_cm.__enter__()
                    cstage["t"] = cs_pool.tile([128, NLT, QW], BF16,
                                               tag="cs", name="cstage")
                    intl = c_gen((0, 1), True)
                    n_per_slot = 1
                elif ui == 14:
                    drain(intl)
                    intl = c_gen((2,), False)
                    n_per_slot = 2
                elif ui == 15:
                    drain(intl)
                    intl = cfinal_gen(range(0, 8), act_half=False, pads=2,
                                      use_av=False)
                    n_per_slot = 2
                intl = emit_unit(ui, intl, n_per_slot)
            # drain last unit's AV, interleaving leftover cfinal steps
            last = len(units) - 1

            def consume(gen, n):
                if gen is None:
                    return None
                with fprio():
                    for _ in range(n):
                        if next(gen, "END") == "END":
                            return None
                return gen

            for j in range(8):
                emit_chain(last, j)
                intl = consume(intl, 4)
                if j == 3:
                    drain(intl)
                    emit_norm(last, 0)   # triggers the lt 8-11 transpose
                    intl = cfinal_gen(range(8, 12), act_half=True, pads=2,
                                      use_av=False)
            emit_norm(last, 1)   # triggers the lt 12-15 transpose
            drain(intl)

            # ---- stage C final pass for the last quarter (critical tail,
            # normal priority) ----
            drain(cfinal_gen(range(12, NLT), act_half=True), filler=False)
            cs_cm.__exit__(None, None, None)

    nc.compile()
    return nc


def _get_nc():
    if "nc" not in _CACHE:
        _CACHE["nc"] = _build()
    return _CACHE["nc"]


def kernel(query, key, value, Wq, bq, Wk, bk, Wv, bv, Wc, bc, **_unused):
    query = np.asarray(query, np.float32)
    key = np.asarray(key, np.float32)
    value = np.asarray(value, np.float32)
    Wq = np.asarray(Wq, np.float32)
    Wk = np.asarray(Wk, np.float32)
    Wv = np.asarray(Wv, np.float32)
    Wc = np.asarray(Wc, np.float32)
    bq = np.asarray(bq, np.float32)
    bk = np.asarray(bk, np.float32)
    bv = np.asarray(bv, np.float32)
    bc = np.asarray(bc, np.float32)

    nc = _get_nc()

    def pack_x(xT):
        # [D, L] -> [128, 4, NI, 512]: per-partition-contiguous 8KB chunks
        return np.ascontiguousarray(
            xT.reshape(NI, 128, 4, 512).transpose(1, 2, 0, 3)).astype(np_bf16)

    def pack_w(WT, c0, c1):
        # [D, cols] slice -> [128, NI, c1-c0], per-partition contiguous
        return np.ascontiguousarray(
            WT[:, c0:c1].reshape(NI, 128, c1 - c0).transpose(1, 0, 2)
        ).astype(np_bf16)

    xtq = [pack_x(query[b].T) for b in range(B)]
    xtk = [pack_x(key[b].T) for b in range(B)]
    xtv = [pack_x(value[b].T) for b in range(B)]
    wq_g = [Wq[g * OC:(g + 1) * OC, :].T for g in range(2)]
    wk_g = [Wk[g * OC:(g + 1) * OC, :].T for g in range(2)]
    wv_g = [Wv[g * OC:(g + 1) * OC, :].T for g in range(2)]
    wct_g = [np.ascontiguousarray(
        Wc[:, g * OC:(g + 1) * OC].T.reshape(NM, 128, D).transpose(1, 0, 2)
    ).astype(np_bf16) for g in range(2)]

    in_maps = []
    for c in range(NCORES):
        b, g = c // 2, c % 2
        in_maps.append({
            "XTQ": xtq[b], "XTK": xtk[b], "XTV": xtv[b],
            "WQH": pack_w(wq_g[g], 0, 128),
            "WQB": pack_w(wq_g[g], 128, OC),
            "WKH": pack_w(wk_g[g], 0, 128),
            "WKB": pack_w(wk_g[g], 128, OC),
            "WVT": pack_w(wv_g[g], 0, OC),
            "WCT": wct_g[g],
            "BQ": np.ascontiguousarray(bq[g * OC:(g + 1) * OC]),
            "BK": np.ascontiguousarray(bk[g * OC:(g + 1) * OC]),
            "BV": np.ascontiguousarray(bv[g * OC:(g + 1) * OC]),
            "EYE": np.eye(128, dtype=np.float32).astype(np_bf16),
        })

    res = run_bass_kernel_spmd(nc, in_maps, core_ids=list(range(NCORES)),
                               **_CACHE.get("run_kwargs", {}))
    _CACHE["last_results"] = res

    outp = np.empty((B, L, D), np.float32)
    for b in range(B):
        outp[b] = (res.results[2 * b]["OUT"].astype(np.float32)
                   + res.results[2 * b + 1]["OUT"].astype(np.float32))
    outp += bc
    return outp


# revision 64
# speedup vs baseline: 1.0033x; 1.0033x over previous
"""Multi-head attention forward on 8 Trainium2 NeuronCores (Bass/Tile).

Problem: B=4, L=2048, D=1024, H=16 heads, DV=64.
  out = softmax((x_q Wq^T)(x_k Wk^T)^T / sqrt(DV)) (x_v Wv^T) Wc^T + biases

Sharding (8 cores): core c handles batch b = c//2 and head-group g = c%2
(8 heads = 512 of the 1024 projection columns). Each core produces a
full-shape [L, D] partial of the output projection; the host sums the two
partials per batch and adds bc.

All device data is bf16 (PSUM accumulation fp32); inputs are converted on
the host. Per-core pipeline:
  A1. V projection -> vext [128, 8, 65] bf16 per l-tile (ones column per
      head gives the softmax denominator through the AV matmul).
  A2. Q/K projections per head-pair -> qt/kt [128(2x64 dims), 2048] bf16.
      Pair-0 head uses dedicated 128-col weight slices so the first
      matmuls only wait on ~3.5MB of DMA (not the bulk weights).
  B.  Per (head, q-half) unit: 16 k-tiles of scores^T [k=128, q=1024] in
      PSUM -> ACT exp (scale=1/8) -> ex bf16; then AV flipped:
      out[q=128, 65] chains (lhsT=ex slice, rhs=vext) -> per-partition
      denominator -> reciprocal_approx_fast + tensor_scalar normalize
      into att_s [128, 64, 16]. Per head-pair, SBUF->SBUF DMA xbar
      transposes produce attnt [64, 2048] (= attn^T) for stage C.
      Pair 3 runs its units qc-major and transposes in q-halves so the
      final output projection can start before the last unit retires.
  C.  Output projection out[l, 1024] accumulated over the 4 head pairs;
      pairs 0-2 staged in bf16, final pass adds pair 3 via an identity
      matmul. lt 0-7 of the final pass interleave into unit 15.
"""

import itertools
from contextlib import ExitStack

import numpy as np

import concourse.bacc as bacc
import concourse.mybir as mybir
from concourse.tile import TileContext
from concourse.bass_utils import run_bass_kernel_spmd

try:
    from ml_dtypes import bfloat16 as np_bf16
except ImportError:  # pragma: no cover
    import jax.numpy as jnp
    np_bf16 = jnp.bfloat16

B, L, D, H = 4, 2048, 1024, 16
DV = 64
HPC = 8           # heads per core
OC = HPC * DV     # 512 projection cols per core
NCORES = 8

F32 = mybir.dt.float32
BF16 = mybir.dt.bfloat16
EXP = mybir.ActivationFunctionType.Exp
MULT = mybir.AluOpType.mult
ADD = mybir.AluOpType.add

NI = D // 128    # 8 contraction tiles for projections
NM = OC // 128   # 4 head pairs
NLT = L // 128   # 16 l/k tiles
QW = 1024        # q-half width in stage B

_CACHE = {}


def _build():
    nc = bacc.Bacc("TRN2", target_bir_lowering=False, debug=False,
                   num_devices=NCORES)

    # Inputs are host-repacked so every DMA descriptor is 2-8KB contiguous
    # per partition (1KB descriptors saturate the 16 queues at half rate):
    #   XT*[p, lc, i, c] = x^T[i*128+p, lc*512+c]
    #   WQH/WKH[p, i, c] = W^T[i*128+p, c]       (head cols 0:128)
    #   WQB/WKB[p, i, c] = W^T[i*128+p, 128+c]   (cols 128:512)
    #   WVT[p, i, c] = Wv^T[i*128+p, c];  WCT[p, m, c] = Wc^T[m*128+p, c]
    xtq = nc.dram_tensor("XTQ", [128, 4, NI, 512], BF16, kind="ExternalInput")
    xtk = nc.dram_tensor("XTK", [128, 4, NI, 512], BF16, kind="ExternalInput")
    xtv = nc.dram_tensor("XTV", [128, 4, NI, 512], BF16, kind="ExternalInput")
    wqh = nc.dram_tensor("WQH", [128, NI, 128], BF16, kind="ExternalInput")
    wqb = nc.dram_tensor("WQB", [128, NI, OC - 128], BF16,
                         kind="ExternalInput")
    wkh = nc.dram_tensor("WKH", [128, NI, 128], BF16, kind="ExternalInput")
    wkb = nc.dram_tensor("WKB", [128, NI, OC - 128], BF16,
                         kind="ExternalInput")
    wvt = nc.dram_tensor("WVT", [128, NI, OC], BF16, kind="ExternalInput")
    wct = nc.dram_tensor("WCT", [128, NM, D], BF16, kind="ExternalInput")
    bqd = nc.dram_tensor("BQ", [OC], F32, kind="ExternalInput")
    bkd = nc.dram_tensor("BK", [OC], F32, kind="ExternalInput")
    bvd = nc.dram_tensor("BV", [OC], F32, kind="ExternalInput")
    eyed = nc.dram_tensor("EYE", [128, 128], BF16, kind="ExternalInput")
    out = nc.dram_tensor("OUT", [L, D], BF16, kind="ExternalOutput")

    # Fillers (projections, C passes) get +1M priority so the greedy tile
    # scheduler only runs them when the exp-paced backbone stalls, instead
    # of monopolizing the PE in long ready-ahead streaks.
    FILLER = -1_000_000

    with TileContext(nc) as tc:
        with (
            tc.tile_pool(name="const", bufs=1) as const_pool,
            tc.tile_pool(name="wcp", bufs=1) as wc_pool,
            tc.tile_pool(name="qkt", bufs=6) as qkt_pool,
            tc.tile_pool(name="vext", bufs=NLT) as vext_pool,
            tc.tile_pool(name="ex", bufs=32) as ex_pool,
            tc.tile_pool(name="atts", bufs=2) as atts_pool,
            tc.tile_pool(name="attnt", bufs=NM) as attnt_pool,
            tc.tile_pool(name="rcp", bufs=4) as rcp_pool,
            tc.tile_pool(name="ob", bufs=4) as ob_pool,
            tc.tile_pool(name="st", bufs=2, space="PSUM") as st_pool,
            tc.tile_pool(name="ap", bufs=2, space="PSUM") as ap_pool,
            tc.tile_pool(name="av", bufs=2, space="PSUM") as av_pool,
        ):
            astack = ExitStack()
            wv_pool = astack.enter_context(tc.tile_pool(name="wvp", bufs=1))
            wq_pool = astack.enter_context(tc.tile_pool(name="wqp", bufs=1))
            wk_pool = astack.enter_context(tc.tile_pool(name="wkp", bufs=1))
            xt_pool = astack.enter_context(tc.tile_pool(name="xt", bufs=4))

            # ---- constants / weights ----
            qb_tile = const_pool.tile([128, NM], F32, tag="bq", name="bqt")
            nc.sync.dma_start(
                out=qb_tile, in_=bqd[:].rearrange("(m p) -> p m", p=128))
            kb_tile = const_pool.tile([128, NM], F32, tag="bk", name="bkt")
            nc.sync.dma_start(
                out=kb_tile, in_=bkd[:].rearrange("(m p) -> p m", p=128))
            vbias = const_pool.tile([128, OC], F32, tag="bv", name="bvt")
            onesf = const_pool.tile([128, HPC], F32, tag="ones", name="ones")
            nc.vector.memset(onesf, 1.0)
            eye = const_pool.tile([128, 128], BF16, tag="eye", name="eye")

            wq_head = wq_pool.tile([128, NI, 128], BF16, tag="wqh",
                                   name="wq_h")
            wq_bulk = wq_pool.tile([128, NI, OC - 128], BF16, tag="wqb",
                                   name="wq_b")
            wk_head = wk_pool.tile([128, NI, 128], BF16, tag="wkh",
                                   name="wk_h")
            wk_bulk = wk_pool.tile([128, NI, OC - 128], BF16, tag="wkb",
                                   name="wk_b")

            def wsel_q(i, m):
                return (wq_head[:, i, :] if m == 0
                        else wq_bulk[:, i, (m - 1) * 128:m * 128])

            def wsel_k(i, m):
                return (wk_head[:, i, :] if m == 0
                        else wk_bulk[:, i, (m - 1) * 128:m * 128])

            wv_all = wv_pool.tile([128, NI, OC], BF16, tag="wv", name="wv_a")
            wv_tiles = [wv_all[:, i, :] for i in range(NI)]
            wc_all = wc_pool.tile([128, NM, D], BF16, tag="wc", name="wc_a")
            wc_tiles = [wc_all[:, i, :] for i in range(NM)]

            # qt/kt allocated lazily from a 6-deep ring: pair 3's tiles reuse
            # pair 0's buffers (dead after unit 3).
            qt = {}
            kt = {}

            def get_qkt(d, m):
                if m not in d:
                    pfx = "qt" if d is qt else "kt"
                    d[m] = qkt_pool.tile([128, L], BF16, tag="qkt",
                                         name=f"{pfx}{m}")
                return d[m]
            vext = [vext_pool.tile([128, HPC, DV + 1], BF16, tag="vext",
                                   name=f"vext{i}")
                    for i in range(NLT)]

            # ---- stage A1 generator: V projection (128 mms + DVE) ----
            # Yields BEFORE each matmul (except the first) so the pull that
            # consumes the last matmul also emits the trailing DVE writes.
            def a1_gen():
                first = [True]

                def tick():
                    if first[0]:
                        first[0] = False
                        return iter(())
                    return iter((None,))

                xas = {}

                def load_v(lc):
                    xa = xt_pool.tile([128, NI, 512], BF16, tag="xt",
                                      name="xv")
                    nc.sync.dma_start(out=xa, in_=xtv[:, lc, :, :])
                    xas[lc] = xa

                load_v(0)
                load_v(1)
                for lc in range(4):
                    if lc + 2 < 4:
                        load_v(lc + 2)
                    xa = xas.pop(lc)
                    for ls in range(4):
                        ps = ap_pool.tile([128, 512], F32, tag="ap",
                                          name="psv")
                        for i in range(NI):
                            yield from tick()
                            nc.tensor.matmul(
                                ps,
                                lhsT=xa[:, i, ls * 128:(ls + 1) * 128],
                                rhs=wv_tiles[i],
                                start=(i == 0), stop=(i == NI - 1))
                        lt = lc * 4 + ls
                        nc.vector.tensor_add(
                            vext[lt][:, :, 0:DV],
                            ps.rearrange("p (h d) -> p h d", h=HPC),
                            vbias.rearrange("p (h d) -> p h d", h=HPC))
                        nc.vector.tensor_copy(vext[lt][:, :, DV], onesf)

            def a2_chunk(wsel, dst, btile, m, xa):
                ps = ap_pool.tile([128, 512], F32, tag="ap", name="psp")
                lc = xa[1]
                for i in range(NI):
                    yield
                    nc.tensor.matmul(
                        ps,
                        lhsT=wsel(i, m),
                        rhs=xa[0][:, i, :],
                        start=(i == 0), stop=(i == NI - 1))
                nc.vector.tensor_scalar(
                    out=dst[:, lc * 512:(lc + 1) * 512],
                    in0=ps,
                    scalar1=btile[:, m:m + 1],
                    scalar2=None, op0=ADD)

            def xload(xsrc, lc, name):
                xa = xt_pool.tile([128, NI, 512], BF16, tag="xt", name=name)
                nc.sync.dma_start(out=xa, in_=xsrc[:, lc, :, :])
                return (xa, lc)

            def run(gen):
                for _ in gen:
                    pass

            tail_xas = []

            def a2_pair0_head():
                # Critical path: only the 128-col (m=0) weight slices +
                # the first three x chunks gate the first matmuls.
                xk0 = xload(xtk, 0, "xk0")
                nc.sync.dma_start(out=wk_head, in_=wkh[:, :, :])
                xq0 = xload(xtq, 0, "xq0")
                nc.sync.dma_start(out=wq_head, in_=wqh[:, :, :])
                xq1 = xload(xtq, 1, "xq1")
                run(a2_chunk(wsel_k, get_qkt(kt, 0), kb_tile, 0, xk0))
                run(a2_chunk(wsel_q, get_qkt(qt, 0), qb_tile, 0, xq0))
                run(a2_chunk(wsel_q, qt[0], qb_tile, 0, xq1))
                # next-critical: the k chunks feeding unit-0 scores k=4..15
                tail_xas.extend(xload(xtk, lc, "xk0t") for lc in (1, 2, 3))
                # bulk weights (cols 128+ needed from ui=2 onward)
                nc.sync.dma_start(out=wk_bulk, in_=wkb[:, :, :])
                nc.sync.dma_start(out=wq_bulk, in_=wqb[:, :, :])

            def a2_pair0_tail():
                first = [True]
                chunks = [(xtk, 1, wsel_k, kt[0], kb_tile),
                          (xtk, 2, wsel_k, kt[0], kb_tile),
                          (xtk, 3, wsel_k, kt[0], kb_tile),
                          (xtq, 2, wsel_q, qt[0], qb_tile),
                          (xtq, 3, wsel_q, qt[0], qb_tile)]
                xas = list(tail_xas)
                for ci, (xsrc, lc, wsel, dst, btile) in enumerate(chunks):
                    if ci + 3 < len(chunks):
                        xas.append(xload(chunks[ci + 3][0],
                                         chunks[ci + 3][1], "xt0t"))
                    g = a2_chunk(wsel, dst, btile, 0, xas[ci])
                    for _ in g:
                        if first[0]:
                            first[0] = False
                        else:
                            yield

            # ---- A2 generator: pair 1 alone, then pairs (2,3) sharing each
            # x chunk so those 16MB of x are loaded once, not twice. ----
            def a2_gen(ms):
                first = [True]

                def tick():
                    if first[0]:
                        first[0] = False
                        return iter(())
                    return iter((None,))

                xas = {}

                def load_x(xsrc, lc):
                    xa = xt_pool.tile([128, NI, 512], BF16, tag="xt",
                                      name="xp")
                    nc.sync.dma_start(out=xa, in_=xsrc[:, lc, :, :])
                    xas[(id(xsrc), lc)] = xa

                # chunk order: q01 (next pair's qc=0 scores), k full (its
                # scores k=0..15 by the pair boundary), q23 (its qc=1 unit).
                chunks = [(xtq, 0), (xtq, 1), (xtk, 0), (xtk, 1),
                          (xtk, 2), (xtk, 3), (xtq, 2), (xtq, 3)]
                for m in ms:
                    get_qkt(qt, m)
                    get_qkt(kt, m)
                load_x(*chunks[0])
                load_x(*chunks[1])
                for ci, (xsrc, lc) in enumerate(chunks):
                    if ci + 2 < len(chunks):
                        load_x(*chunks[ci + 2])
                    if ci == 5 and 3 in ms:
                        nc.sync.dma_start(out=wc_all, in_=wct[:, :, :])
                    xa = xas.pop((id(xsrc), lc))
                    wsel = wsel_q if xsrc is xtq else wsel_k
                    dst_l = qt if xsrc is xtq else kt
                    btile = qb_tile if xsrc is xtq else kb_tile
                    for m in ms:
                        ps = ap_pool.tile([128, 512], F32, tag="ap",
                                          name="psp")
                        for i in range(NI):
                            yield from tick()
                            nc.tensor.matmul(
                                ps,
                                lhsT=wsel(i, m),
                                rhs=xa[:, i, :],
                                start=(i == 0), stop=(i == NI - 1))
                        nc.vector.tensor_scalar(
                            out=dst_l[m][:, lc * 512:(lc + 1) * 512],
                            in0=ps,
                            scalar1=btile[:, m:m + 1],
                            scalar2=None, op0=ADD)

            def fprio():
                return tc.high_priority(offset=FILLER)

            def drain(gen, filler=True):
                if gen is None:
                    return
                if filler:
                    with fprio():
                        for _ in gen:
                            pass
                else:
                    for _ in gen:
                        pass

            # ---- pre-B: A2(0) head, then non-critical consts + V weights ----
            a2_pair0_head()
            nc.sync.dma_start(
                out=vbias, in_=bvd[:].unsqueeze(0).to_broadcast((128, OC)))
            nc.sync.dma_start(out=eye, in_=eyed[:, :])
            nc.sync.dma_start(out=wv_all, in_=wvt[:, :, :])

            # ---- stage B ----
            # Units: (m, h2, qc); AV of unit u runs after scores of u+1.
            # Pair 3 runs qc-major so the q-halves of attnt[3] complete (and
            # transpose) separately, letting stage C start before the end.
            units = []
            for m in range(NM):
                order = (((0, 0), (0, 1), (1, 0), (1, 1)) if m < 3
                         else ((0, 0), (1, 0), (0, 1), (1, 1)))
                for h2, qc in order:
                    units.append((m, h2, qc))

            cstage = {}     # opened after stage-A pools close

            def c_gen(dts, first):
                # stage-C partial pass: psum chain over `dts`, then DVE
                # copy/accumulate into the bf16 staging tile.
                firstmm = [True]

                def tick():
                    if firstmm[0]:
                        firstmm[0] = False
                        return iter(())
                    return iter((None,))

                for lt in range(NLT):
                    for nck in range(2):
                        ps = ap_pool.tile([128, 512], F32, tag="ap",
                                          name="psc")
                        for di, dt in enumerate(dts):
                            yield from tick()
                            nc.tensor.matmul(
                                ps,
                                lhsT=attnt[dt][:, lt * 128:(lt + 1) * 128],
                                rhs=wc_tiles[dt][:, nck * 512:(nck + 1) * 512],
                                start=(di == 0), stop=(di == len(dts) - 1))
                        sl = cstage["t"][:, lt, nck * 512:(nck + 1) * 512]
                        if first:
                            nc.vector.tensor_copy(sl, ps)
                        else:
                            nc.vector.tensor_add(sl, ps, sl)

            def cfinal_gen(lts, act_half=False, pads=0, use_av=True):
                # final stage-C pass: pair 3 + staged partial via identity
                # matmul; PSUM drained by DVE (and ACT when it is free).
                # use_av=False keeps it off av_pool while AV tiles of the
                # last units still occupy that ring (circular-wait hazard).
                for _ in range(pads):
                    yield
                for lt in lts:
                    ob = ob_pool.tile([128, QW], BF16, tag="ob", name="ob")
                    for nck in range(2):
                        pool, tg = ((ap_pool, "ap") if (nck == 0 or not use_av)
                                    else (av_pool, "av"))
                        ps = pool.tile([128, 512], F32, tag=tg, name="psf")
                        nc.tensor.matmul(
                            ps,
                            lhsT=attnt[3][:, lt * 128:(lt + 1) * 128],
                            rhs=wc_tiles[3][:, nck * 512:(nck + 1) * 512],
                            start=True, stop=False)
                        yield
                        nc.tensor.matmul(
                            ps,
                            lhsT=eye,
                            rhs=cstage["t"][:, lt, nck * 512:(nck + 1) * 512],
                            start=False, stop=True)
                        if nck == 0:
                            nc.vector.tensor_copy(ob[:, 0:512], ps)
                        elif act_half:
                            nc.scalar.copy(ob[:, 512:QW], ps)
                        else:
                            nc.vector.tensor_copy(ob[:, 512:QW], ps)
                        yield
                    nc.sync.dma_start(
                        out=out[lt * 128:(lt + 1) * 128, :], in_=ob)

            exs = {}        # unit idx -> list of 16 ex tiles
            att_s = {}      # pair -> staging tile
            avt = {}        # (ui, qsg) -> av psum region view
            attnt = {}

            def emit_chain(pu, j):
                # one AV accumulation chain (16 k-steps) for q-subtile j.
                # Chains are sequential per PSUM bank: interleaving open
                # accumulation groups within a bank clobbers earlier regions.
                m, h2, qc = units[pu]
                h = m * 2 + h2
                qsg, qs = j // 4, j % 4
                if (pu, qsg) not in avt:
                    t = av_pool.tile([128, 512], F32, tag="av", name="av")
                    avt[(pu, qsg)] = t[:, 0:4 * (DV + 1)].rearrange(
                        "p (a b) -> p a b", b=DV + 1)
                av = avt[(pu, qsg)]
                ex_list = exs[pu]
                qq = (qsg * 4 + qs) * 128
                for k in range(NLT):
                    nc.tensor.matmul(
                        av[:, qs, :],
                        lhsT=ex_list[k][:, qq:qq + 128],
                        rhs=vext[k][:, h, :],
                        start=(k == 0), stop=(k == NLT - 1))

            def emit_norm(pu, qsg):
                m, h2, qc = units[pu]
                av = avt.pop((pu, qsg))
                ast = att_s[m]
                rc = rcp_pool.tile([128, 4], F32, tag="rc", name="rc")
                nc.vector.reciprocal_approx_fast(out=rc, in_=av[:, :, DV])
                for qs in range(4):
                    lt = qc * 8 + qsg * 4 + qs
                    nc.vector.tensor_scalar(
                        out=ast[:, lt, h2 * DV:(h2 + 1) * DV],
                        in0=av[:, qs, 0:DV],
                        scalar1=rc[:, qs:qs + 1],
                        scalar2=None, op0=MULT)
                if m == 3 and h2 == 1:
                    # qc-major order: 4-lt quarter of pair 3 complete; early
                    # transposes let the final C pass chase the AV drain.
                    if 3 not in attnt:
                        attnt[3] = attnt_pool.tile(
                            [128, L], BF16, tag="attnt", name="attnt3")
                    lt0 = qc * 8 + qsg * 4
                    dst = attnt[3][:, lt0 * 128:(lt0 + 4) * 128].rearrange(
                        "p (t q) -> p t q", t=4)
                    nc.sync.dma_start_transpose(
                        dst, ast[:, lt0:lt0 + 4, :])
                    if qc == 1 and qsg == 1:
                        del att_s[3]
                if qsg == 1:
                    exs.pop(pu)
                    if m < 3 and h2 == 1 and qc == 1:
                        # pair complete: xbar transpose
                        # out[fm, b, q] = in[q, b*128+fm], fm = h2*64+dv
                        attnt[m] = attnt_pool.tile(
                            [128, L], BF16, tag="attnt", name=f"attnt{m}")
                        dst = attnt[m][:, :].rearrange(
                            "p (t q) -> p t q", t=NLT)
                        nc.sync.dma_start_transpose(dst, ast)
                        del att_s[m]

            # prev-unit AV schedules: slot -> actions (c<j> chain, n<g> norm)
            # Nothing at "end": bunching c7+n1 there delays the next unit's
            # first scores and opens a ~1.2us ACT bubble per unit boundary.
            SPREAD = {1: ["c0"], 3: ["c1"], 5: ["c2"], 7: ["c3"],
                      8: ["n0"], 9: ["c4"], 11: ["c5"], 13: ["c6"],
                      15: ["c7", "n1"]}
            SQUEEZE = {10: ["c0"], 11: ["c1"], 12: ["c2", "c3"],
                       13: ["n0", "c4"], 14: ["c5", "c6"],
                       15: ["c7", "n1"]}

            def do_actions(prev, acts):
                for a in acts:
                    if a[0] == "c":
                        emit_chain(prev, int(a[1]))
                    else:
                        emit_norm(prev, int(a[1]))

            def emit_unit(ui, intl_gen, n_intl):
                m, h2, qc = units[ui]
                off = h2 * DV
                prev = ui - 1 if ui > 0 else None
                sched = SQUEEZE if ui == 1 else SPREAD
                if m not in att_s:
                    att_s[m] = atts_pool.tile(
                        [128, NLT, 128], BF16, tag="atts", name=f"atts{m}")
                ex_list = []
                exs[ui] = ex_list
                for k in range(NLT):
                    if intl_gen is not None:
                        with fprio():
                            for _ in range(n_intl):
                                if next(intl_gen, "END") == "END":
                                    intl_gen = None
                                    break
                    if prev is not None:
                        do_actions(prev, sched.get(k, ()))
                    st = st_pool.tile([128, QW], F32, tag="st", name="st")
                    for j in range(2):
                        nc.tensor.matmul(
                            st[:, j * 512:(j + 1) * 512],
                            lhsT=kt[m][off:off + DV, k * 128:(k + 1) * 128],
                            rhs=qt[m][off:off + DV,
                                      qc * QW + j * 512:qc * QW + (j + 1) * 512],
                            start=True, stop=True)
                    ex = ex_pool.tile([128, QW], BF16, tag="ex", name="ex")
                    nc.scalar.activation(out=ex, in_=st, func=EXP, scale=0.125)
                    ex_list.append(ex)
                if prev is not None:
                    do_actions(prev, sched.get("end", ()))
                return intl_gen

            intl = itertools.chain(a2_pair0_tail(), a1_gen())
            n_per_slot = 8
            for ui in range(len(units)):
                if ui == 2:
                    drain(intl)
                    intl = a2_gen((1,))
                    n_per_slot = 2
                elif ui == 4:
                    drain(intl)
                    intl = a2_gen((2, 3))
                    n_per_slot = 2
                elif ui == 8:
                    drain(intl)
                    intl = None
                elif ui == 10:
                    astack.close()
                    cs_cm = tc.tile_pool(name="cst", bufs=1)
                    cs_pool = cs_cm.__enter__()
                    cstage["t"] = cs_pool.tile([128, NLT, QW], BF16,
                                               tag="cs", name="cstage")
                    intl = c_gen((0, 1), True)
                    n_per_slot = 1
                elif ui == 14:
                    drain(intl)
                    intl = c_gen((2,), False)
                    n_per_slot = 2
                elif ui == 15:
                    drain(intl)
                    intl = cfinal_gen(range(0, 8), act_half=False, pads=2,
                                      use_av=False)
                    n_per_slot = 2
                intl = emit_unit(ui, intl, n_per_slot)
            # drain last unit's AV, interleaving leftover cfinal steps
            last = len(units) - 1

            def consume(gen, n):
                if gen is None:
                    return None
                with fprio():
                    for _ in range(n):
                        if next(gen, "END") == "END":
                            return None
                return gen

            for j in range(8):
                emit_chain(last, j)
                intl = consume(intl, 4)
                if j == 3:
                    drain(intl)
                    emit_norm(last, 0)   # triggers the lt 8-11 transpose
                    intl = cfinal_gen(range(8, 12), act_half=True, pads=2,
                                      use_av=False)
            emit_norm(last, 1)   # triggers the lt 12-15 transpose
            drain(intl)

            # ---- stage C final pass for the last quarter (critical tail,
            # normal priority) ----
            drain(cfinal_gen(range(12, NLT), act_half=True), filler=False)
            cs_cm.__exit__(None, None, None)

    nc.compile()
    return nc


def _get_nc():
    if "nc" not in _CACHE:
        _CACHE["nc"] = _build()
    return _CACHE["nc"]


def kernel(query, key, value, Wq, bq, Wk, bk, Wv, bv, Wc, bc, **_unused):
    query = np.asarray(query, np.float32)
    key = np.asarray(key, np.float32)
    value = np.asarray(value, np.float32)
    Wq = np.asarray(Wq, np.float32)
    Wk = np.asarray(Wk, np.float32)
    Wv = np.asarray(Wv, np.float32)
    Wc = np.asarray(Wc, np.float32)
    bq = np.asarray(bq, np.float32)
    bk = np.asarray(bk, np.float32)
    bv = np.asarray(bv, np.float32)
    bc = np.asarray(bc, np.float32)

    nc = _get_nc()

    def pack_x(xT):
        # [D, L] -> [128, 4, NI, 512]: per-partition-contiguous 8KB chunks
        return np.ascontiguousarray(
            xT.reshape(NI, 128, 4, 512).transpose(1, 2, 0, 3)).astype(np_bf16)

    def pack_w(WT, c0, c1):
        # [D, cols] slice -> [128, NI, c1-c0], per-partition contiguous
        return np.ascontiguousarray(
            WT[:, c0:c1].reshape(NI, 128, c1 - c0).transpose(1, 0, 2)
        ).astype(np_bf16)

    xtq = [pack_x(query[b].T) for b in range(B)]
    xtk = [pack_x(key[b].T) for b in range(B)]
    xtv = [pack_x(value[b].T) for b in range(B)]
    wq_g = [Wq[g * OC:(g + 1) * OC, :].T for g in range(2)]
    wk_g = [Wk[g * OC:(g + 1) * OC, :].T for g in range(2)]
    wv_g = [Wv[g * OC:(g + 1) * OC, :].T for g in range(2)]
    wct_g = [np.ascontiguousarray(
        Wc[:, g * OC:(g + 1) * OC].T.reshape(NM, 128, D).transpose(1, 0, 2)
    ).astype(np_bf16) for g in range(2)]

    in_maps = []
    for c in range(NCORES):
        b, g = c // 2, c % 2
        in_maps.append({
            "XTQ": xtq[b], "XTK": xtk[b], "XTV": xtv[b],
            "WQH": pack_w(wq_g[g], 0, 128),
            "WQB": pack_w(wq_g[g], 128, OC),
            "WKH": pack_w(wk_g[g], 0, 128),
            "WKB": pack_w(wk_g[g], 128, OC),
            "WVT": pack_w(wv_g[g], 0, OC),
            "WCT": wct_g[g],
            "BQ": np.ascontiguousarray(bq[g * OC:(g + 1) * OC]),
            "BK": np.ascontiguousarray(bk[g * OC:(g + 1) * OC]),
            "BV": np.ascontiguousarray(bv[g * OC:(g + 1) * OC]),
            "EYE": np.eye(128, dtype=np.float32).astype(np_bf16),
        })

    res = run_bass_kernel_spmd(nc, in_maps, core_ids=list(range(NCORES)),
                               **_CACHE.get("run_kwargs", {}))
    _CACHE["last_results"] = res

    outp = np.empty((B, L, D), np.float32)
    for b in range(B):
        outp[b] = (res.results[2 * b]["OUT"].astype(np.float32)
                   + res.results[2 * b + 1]["OUT"].astype(np.float32))
    outp += bc
    return outp
